# revision 15
# baseline (speedup 1.0000x reference)
"""AGATCellWithMLP Trainium2 kernel: 8-core data-parallel over batch B.

Each core handles one graph. Host-side we permute each graph's nodes so the
512 "selected" nodes (nodes_n order) come first; the kernel then only computes
attention/MLP query rows for those 512 nodes (keys/values span all 1024).
All heavy tensors are kept in transposed [channel, node] layout so the
attention probabilities never need an on-chip transpose; the softmax
denominator is produced by an extra ones-column in the V matmul.

The combined-feature dim C=129 is split as 128 PE-contraction rows plus a
rank-1 "channel 128" correction (fused DVE op or tiny extra K-chunk), so
every big matmul needs only ONE K-chunk instead of two.
"""

import sys

sys.path.insert(0, "/opt/trn_rl_repo")

from contextlib import ExitStack

import numpy as np
import ml_dtypes

import concourse.bass as bass
import concourse.bacc as bacc
import concourse.tile as tile
from concourse import mybir
from concourse.bass_utils import run_bass_kernel_spmd
from concourse.masks import make_identity
from concourse.bass import ts

P = 128
B, N, D, H, QV = 8, 1024, 64, 4, 32
C = 2 * D + 1           # 129
KD = C // 8             # 16
NQ = 512                # selected nodes (queries) per graph
CX, CH = D + 1, D       # 65 + 64 channel split (x | h)
NEG = -9e15
F32 = mybir.dt.float32
BF16 = mybir.dt.bfloat16
AX = mybir.AxisListType
ALU = mybir.AluOpType
ACTF = mybir.ActivationFunctionType

NT = N // P             # 8 key tiles
MT = NQ // P            # 4 query tiles


def build_graph():
    nc = bacc.Bacc()

    xp = nc.declare_dram_parameter("xp", [N, CX], F32, False)
    hp = nc.declare_dram_parameter("hp", [N, CH], F32, False)
    adjT = nc.declare_dram_parameter("adjT", [N, NQ], F32, False)
    qv = nc.declare_dram_parameter("qv", [NQ, QV], F32, False)
    # per-head q|k weights packed [C, 48]: k at cols 0:16, q (pre-scaled) at 32:48
    wqk = nc.declare_dram_parameter("wqk", [H, C, 48], BF16, False)
    bqk = nc.declare_dram_parameter("bqk", [H, 48], F32, False)
    wv = nc.declare_dram_parameter("wv", [H, C, C], BF16, False)
    w1 = nc.declare_dram_parameter("w1", [H, C, C], BF16, False)
    b1 = nc.declare_dram_parameter("b1", [C], F32, False)
    w2 = nc.declare_dram_parameter("w2", [C, C], BF16, False)
    b2 = nc.declare_dram_parameter("b2", [C], F32, False)
    wall = nc.declare_dram_parameter("wall", [3, P, QV * D], BF16, False)
    w128 = nc.declare_dram_parameter("w128", [3, QV, D], BF16, False)
    ball = nc.declare_dram_parameter("ball", [3, QV, D], BF16, False)
    out = nc.declare_dram_parameter("out", [NQ, D], F32, True)

    with tile.TileContext(nc) as tc, ExitStack() as ctx:
        sing = ctx.enter_context(tc.tile_pool(name="sing", bufs=1))
        work = ctx.enter_context(tc.tile_pool(name="work", bufs=3))
        pexp_pool = ctx.enter_context(tc.tile_pool(name="pexp", bufs=3))
        vpool = ctx.enter_context(tc.tile_pool(name="vpool", bufs=3))
        psA = ctx.enter_context(tc.tile_pool(name="psA", bufs=3, space="PSUM"))
        psB = ctx.enter_context(tc.tile_pool(name="psB", bufs=2, space="PSUM"))
        psC = ctx.enter_context(tc.tile_pool(name="psC", bufs=1, space="PSUM"))
        psS = ctx.enter_context(tc.tile_pool(name="psS", bufs=2, space="PSUM"))

        def pA(p_, f_):
            return psA.tile([p_, f_], F32, tag="pA", name="pA")

        def pAb(p_, f_):
            return psA.tile([p_, f_], BF16, tag="pA", name="pA")

        def pB(p_, f_):
            return psB.tile([p_, f_], F32, tag="pB", name="pB")

        def pC(p_, f_):
            return psC.tile([p_, f_], F32, tag="pC", name="pC")

        def pS(p_, f_):
            return psS.tile([p_, f_], F32, tag="pS", name="pS")

        ident = sing.tile([P, P], BF16)
        make_identity(nc, ident[:])
        identf = sing.tile([P, P], F32)
        make_identity(nc, identf[:])
        ones_row = sing.tile([1, P], BF16)
        nc.vector.memset(ones_row[:], 1.0)

        # ---- inputs on the gpsimd queue (parallel with weight DMAs) ----
        comb = [sing.tile([P, C], F32, tag=f"comb{i}", name=f"comb{i}")
                for i in range(NT)]
        for i in range(NT):
            nc.gpsimd.dma_start(comb[i][:, 0:CX], xp[ts(i, P), :])
            nc.gpsimd.dma_start(comb[i][:, CX:C], hp[ts(i, P), :])
        qv_sb = [sing.tile([P, QV], F32, tag=f"qv{j}", name=f"qv{j}")
                 for j in range(MT)]
        for j in range(MT):
            nc.gpsimd.dma_start(qv_sb[j][:], qv[ts(j, P), :])
        adjT_sb = [sing.tile([P, NQ], F32, tag=f"adjT{i}", name=f"adjT{i}")
                   for i in range(NT)]
        for i in range(NT):
            nc.gpsimd.dma_start(adjT_sb[i][:], adjT[ts(i, P), :])

        # ---- weights on the sync queue ----
        wqk_sb = [sing.tile([P, 48], BF16, tag=f"wqk{h}", name=f"wqk{h}")
                  for h in range(H)]
        wqkL = [sing.tile([1, 48], BF16, tag=f"wqkL{h}", name=f"wqkL{h}")
                for h in range(H)]
        wv_sb = [sing.tile([P, C], BF16, tag=f"wv{h}", name=f"wv{h}")
                 for h in range(H)]
        wvL = [sing.tile([1, C], BF16, tag=f"wvL{h}", name=f"wvL{h}")
               for h in range(H)]
        w1_sb = [sing.tile([P, C], BF16, tag=f"w1{h}", name=f"w1{h}")
                 for h in range(H)]
        w1L = sing.tile([H, C], BF16, name="w1L")
        for h in range(H):
            nc.sync.dma_start(wqk_sb[h][:], wqk[h, 0:P, :])
            nc.sync.dma_start(wqkL[h][:], wqk[h, P:C, :])
            nc.sync.dma_start(wv_sb[h][:], wv[h, 0:P, :])
            nc.sync.dma_start(wvL[h][:], wv[h, P:C, :])
            nc.sync.dma_start(w1_sb[h][:], w1[h, 0:P, :])
            nc.sync.dma_start(w1L[h:h + 1, :], w1[h, P:C, :])
        w2_sb = (sing.tile([CX, C], BF16, tag="w2x", name="w2x"),
                 sing.tile([CH, C], BF16, tag="w2h", name="w2h"))
        nc.sync.dma_start(w2_sb[0][:], w2[0:CX, :])
        nc.sync.dma_start(w2_sb[1][:], w2[CX:C, :])
        bqk_sb = [sing.tile([48, 1], F32, tag=f"bqk{h}", name=f"bqk{h}")
                  for h in range(H)]
        for h in range(H):
            nc.sync.dma_start(bqk_sb[h][:], bqk[h, :, None])
        b1_sb = (sing.tile([CX, 1], F32, tag="b1x", name="b1x"),
                 sing.tile([CH, 1], F32, tag="b1h", name="b1h"))
        nc.sync.dma_start(b1_sb[0][:], b1[0:CX, None])
        nc.sync.dma_start(b1_sb[1][:], b1[CX:C, None])
        b2_sb = (sing.tile([CX, 1], F32, tag="b2x", name="b2x"),
                 sing.tile([CH, 1], F32, tag="b2h", name="b2h"))
        nc.sync.dma_start(b2_sb[0][:], b2[0:CX, None])
        nc.sync.dma_start(b2_sb[1][:], b2[CX:C, None])
        wall_sb = [sing.tile([P, QV * D], BF16, tag=f"wall{i}", name=f"wall{i}")
                   for i in range(3)]
        w128_sb = [sing.tile([QV, D], BF16, tag=f"w128_{i}", name=f"w128_{i}")
                   for i in range(3)]
        ball_sb = [sing.tile([QV, D], BF16, tag=f"ball{i}", name=f"ball{i}")
                   for i in range(3)]
        for i in range(3):
            nc.sync.dma_start(wall_sb[i][:], wall[i, :, :])
            nc.sync.dma_start(w128_sb[i][:], w128[i, :, :])
            nc.sync.dma_start(ball_sb[i][:], ball[i, :, :])

        # ---- transposed combined: combT [128, 1024] (c 0:128) + c128T [1, 1024]
        combT = sing.tile([P, N], BF16)
        c128T = sing.tile([1, N], BF16)
        for i in range(NT):
            cb = work.tile([P, C], BF16, tag="cb", name="cb")
            nc.scalar.copy(cb[:], comb[i][:])
            pt = pAb(P, P)
            nc.tensor.transpose(pt[:], cb[:, 0:P], ident[:])
            nc.scalar.copy(combT[:, ts(i, P)], pt[:])
            pl = pAb(1, P)
            nc.tensor.transpose(pl[:], cb[:, P:C], ident[:])
            nc.scalar.copy(c128T[:, ts(i, P)], pl[:])
        # qv transposed [32, 512]
        qvT = sing.tile([QV, NQ], BF16)
        for j in range(MT):
            qb16 = work.tile([P, QV], BF16, tag="qb16", name="qb16")
            nc.scalar.copy(qb16[:], qv_sb[j][:])
            pq = pAb(QV, P)
            nc.tensor.transpose(pq[:], qb16[:], ident[:])
            nc.scalar.copy(qvT[:, ts(j, P)], pq[:])

        # residual rows for the cf h-group: channels 65:129 = combT[65:128]+c128T
        combresH = sing.tile([CH, NQ], BF16)
        nc.gpsimd.dma_start(combresH[0:CH - 1, :], combT[CX:P, 0:NQ])
        nc.gpsimd.dma_start(combresH[CH - 1:CH, :], c128T[:, 0:NQ])

        # ---- attention heads ----
        acT = []
        for h in range(H):
            # q,k in one matmul pair: psum rows 0:16 = k, rows 32:48 = q
            kT = work.tile([KD, N], BF16, tag="kT", name="kT")
            qT = work.tile([KD, NQ], BF16, tag="qT", name="qT")
            for half in range(2):
                pk = pC(48, NQ)
                nc.tensor.matmul(pk[:], wqk_sb[h][:], combT[:, ts(half, NQ)],
                                 start=True, stop=False)
                nc.tensor.matmul(pk[:], wqkL[h][:], c128T[:, ts(half, NQ)],
                                 start=False, stop=True)
                nc.scalar.activation(kT[:, ts(half, NQ)], pk[0:KD, :],
                                     ACTF.Identity, bias=bqk_sb[h][0:KD, :])
                if half == 0:
                    nc.scalar.activation(qT[:], pk[32:48, :], ACTF.Identity,
                                         bias=bqk_sb[h][32:48, :])

            # broadcast Wv's channel-128 row across partitions (for the V fixup)
            pvl = pA(P, C)
            nc.tensor.matmul(pvl[:], ones_row[:], wvL[h][:], start=True, stop=True)
            wvLrep = work.tile([P, C], BF16, tag="wvLrep", name="wvLrep")
            nc.vector.tensor_copy(wvLrep[:], pvl[:])

            phg1 = pB(CX, NQ)
            phg2 = pB(CX, NQ)
            for i in range(NT):
                # V tile [128, 130]: cols 0:129 = combined @ Wv, col 129 = 1.0
                vt = vpool.tile([P, C + 1], BF16, tag="vt", name="vt")
                pv = pA(P, C)
                nc.tensor.matmul(pv[:], combT[:, ts(i, P)], wv_sb[h][:],
                                 start=True, stop=True)
                # channel-128 rank-1 fixup fused with the PSUM->SBUF copy
                nc.vector.scalar_tensor_tensor(
                    vt[:, 0:C], wvLrep[:], comb[i][:, P:C], pv[:],
                    op0=ALU.mult, op1=ALU.add)
                nc.vector.memset(vt[:, C:C + 1], 1.0)

                # scores^T tile: [128 keys, 512 queries]
                ps = pS(P, NQ)
                nc.tensor.matmul(ps[:], kT[:, ts(i, P)], qT[:],
                                 start=True, stop=True)
                # mask first (leaky(s+m) == leaky(s)+m for m in {0, -9e15}),
                # then leaky = max(t, 0.2t) in one fused DVE op, then exp.
                t0 = work.tile([P, NQ], F32, tag="t0", name="t0")
                nc.vector.tensor_tensor(t0[:], ps[:], adjT_sb[i][:], ALU.add)
                sm = work.tile([P, NQ], F32, tag="sm", name="sm")
                nc.vector.scalar_tensor_tensor(sm[:], t0[:], 0.2, t0[:],
                                               op0=ALU.mult, op1=ALU.max)
                pe = pexp_pool.tile([P, NQ], BF16, tag="pe", name="pe")
                nc.scalar.activation(pe[:], sm[:], ACTF.Exp)

                nc.tensor.matmul(phg1[:], vt[:, 0:CX], pe[:],
                                 start=(i == 0), stop=(i == NT - 1))
                nc.tensor.matmul(phg2[:], vt[:, CX:C + 1], pe[:],
                                 start=(i == 0), stop=(i == NT - 1))

            # stash raw hp (and the denominator row); normalization is batched
            ar1 = sing.tile([CX, NQ], F32, tag=f"araw{h}x", name=f"araw{h}x")
            ar2 = sing.tile([CX, NQ], F32, tag=f"araw{h}h", name=f"araw{h}h")
            nc.scalar.copy(ar1[:], phg1[:])
            nc.scalar.copy(ar2[:], phg2[:])
            acT.append((ar1, ar2))

        # ---- batched softmax normalization: ONE reciprocal for all 4 heads.
        # Engine reads/writes need base partition 0/32/64, so row scatter and
        # gather go through tiny SBUF-SBUF DMAs.
        rcat = sing.tile([H, NQ], F32, name="rcat")
        for h in range(H):
            nc.gpsimd.dma_start(rcat[h:h + 1, :], acT[h][1][CX - 1:CX, :])
        rinv4 = sing.tile([H, NQ], F32, name="rinv4")
        nc.vector.reciprocal(rinv4[:], rcat[:])
        rtmp = [sing.tile([1, NQ], F32, tag=f"rtmp{h}", name=f"rtmp{h}")
                for h in range(H)]
        for h in range(H):
            nc.gpsimd.dma_start(rtmp[h][:], rinv4[h:h + 1, :])
        # normalized attn_cat^T per head: a128 [128, 512] (c 0:128) in one tile
        # (rows 65:128 arrive via DMA), last channels gathered into aL [4, 512]
        a128 = []
        aL = sing.tile([H, NQ], BF16, name="aL")
        for h in range(H):
            ar1, ar2 = acT[h]
            rinv_bf = work.tile([1, NQ], BF16, tag="rinv_bf", name="rinv_bf")
            nc.scalar.copy(rinv_bf[:], rtmp[h][:])
            pbc = pA(P, NQ)
            nc.tensor.matmul(pbc[:], ones_row[:], rinv_bf[:], start=True, stop=True)
            rb = work.tile([P, NQ], F32, tag="rb", name="rb")
            nc.scalar.copy(rb[:], pbc[:])
            ah = sing.tile([P, NQ], BF16, tag=f"a128_{h}", name=f"a128_{h}")
            nc.vector.tensor_tensor(ah[0:CX, :], ar1[:], rb[0:CX, :], ALU.mult)
            a2t = sing.tile([CH, NQ], BF16, tag=f"a2t{h}", name=f"a2t{h}")
            nc.vector.tensor_tensor(a2t[:], ar2[0:CH, :], rb[0:CH, :], ALU.mult)
            nc.gpsimd.dma_start(ah[CX:P, :], a2t[0:CH - 1, :])
            nc.gpsimd.dma_start(aL[h:h + 1, :], a2t[CH - 1:CH, :])
            a128.append(ah)

        # ---- MLP (transposed): m1 = relu(W1^T ac + b1); cf = W2^T m1 + b2 + comb
        m1T = (work.tile([CX, NQ], BF16, tag="m1x", name="m1x"),
               work.tile([CH, NQ], BF16, tag="m1h", name="m1h"))
        for g, (off, ln) in enumerate(((0, CX), (CX, CH))):
            pm = pB(CX, NQ)
            for h in range(H):
                nc.tensor.matmul(pm[0:ln, :], w1_sb[h][:, off:off + ln],
                                 a128[h][:], start=(h == 0), stop=False)
            nc.tensor.matmul(pm[0:ln, :], w1L[:, off:off + ln], aL[:],
                             start=False, stop=True)
            nc.scalar.activation(m1T[g][:], pm[0:ln, :], ACTF.Relu, bias=b1_sb[g][:])
        # cf128 [128, 512] = combined_final^T channels 0:128; cl_row = channel 128
        cf128 = sing.tile([P, NQ], BF16, name="cf128")
        cfh = work.tile([CH, NQ], BF16, tag="cfh", name="cfh")
        for g, (off, ln) in enumerate(((0, CX), (CX, CH))):
            pm = pB(CX, NQ)
            nc.tensor.matmul(pm[0:ln, :], w2_sb[0][:, off:off + ln], m1T[0][:],
                             start=True, stop=False)
            nc.tensor.matmul(pm[0:ln, :], w2_sb[1][:, off:off + ln], m1T[1][:],
                             start=False, stop=True)
            dst = cf128[0:CX, :] if g == 0 else cfh[:]
            src = combT[0:CX, 0:NQ] if g == 0 else combresH[:]
            nc.vector.scalar_tensor_tensor(dst, pm[0:ln, :], b2_sb[g][:], src,
                                           op0=ALU.add, op1=ALU.add)
        nc.gpsimd.dma_start(cf128[CX:P, :], cfh[0:CH - 1, :])
        cl_row = sing.tile([1, NQ], BF16, name="cl_row")
        nc.gpsimd.dma_start(cl_row[:], cfh[CH - 1:CH, :])
        # natural-layout channel-128 scalars [128, 1] per query tile
        scl = [sing.tile([P, 1], F32, tag=f"scl{j}", name=f"scl{j}")
               for j in range(MT)]
        for j in range(MT):
            pt = pAb(P, 1)
            nc.tensor.transpose(pt[:], cl_row[:, ts(j, P)], ident[0:1, 0:1])
            nc.scalar.copy(scl[j][:], pt[:])

        # ---- hypernetwork stage ----
        def hyper(idx, sel128, selL, func, outs):
            """outs[j][128,64] = func(sel @ W_all + selL*(qv@W128) + qv @ b).

            wall[c, o*QV+d] = W[d, c, o] for c<128; the c=128 row is handled
            via the per-partition scalar selL and qv @ W128.
            """
            og = NQ // QV  # o-values per 512-wide chunk (16)
            for j in range(MT):
                pbn = pC(P, 2 * D)
                nc.tensor.matmul(pbn[:, 0:D], qvT[:, ts(j, P)], ball_sb[idx][:],
                                 start=True, stop=True)
                nc.tensor.matmul(pbn[:, D:2 * D], qvT[:, ts(j, P)],
                                 w128_sb[idx][:], start=True, stop=True)
                o1 = work.tile([P, D], F32, tag="o1", name="o1")
                for nch in range(4):  # 512-wide chunks of the 2048 (o,d) axis
                    pt = pA(P, NQ)
                    nc.tensor.matmul(pt[:], sel128[:, ts(j, P)],
                                     wall_sb[idx][:, ts(nch, NQ)],
                                     start=True, stop=True)
                    prod = work.tile([P, NQ], F32, tag="prod", name="prod")
                    qb = qv_sb[j][:, None, :].to_broadcast((P, og, QV))
                    nc.vector.tensor_tensor(
                        prod[:].rearrange("p (a b) -> p a b", b=QV),
                        pt[:].rearrange("p (a b) -> p a b", b=QV),
                        qb, ALU.mult)
                    nc.vector.tensor_reduce(
                        o1[:, ts(nch, og)],
                        prod[:].rearrange("p (a b) -> p a b", b=QV),
                        axis=AX.X, op=ALU.add)
                nc.vector.tensor_tensor(o1[:], o1[:], pbn[:, 0:D], ALU.add)
                nc.vector.scalar_tensor_tensor(o1[:], pbn[:, D:2 * D], selL[j],
                                               o1[:], op0=ALU.mult, op1=ALU.add)
                nc.scalar.activation(outs[j][:], o1[:], func)

        r_t = [work.tile([P, D], F32, tag=f"r{j}", name=f"r{j}") for j in range(MT)]
        u_t = [work.tile([P, D], F32, tag=f"u{j}", name=f"u{j}") for j in range(MT)]
        c_t = [work.tile([P, D], F32, tag=f"c{j}", name=f"c{j}") for j in range(MT)]
        hyper(0, cf128, [scl[j][:] for j in range(MT)], ACTF.Sigmoid, r_t)
        hyper(1, cf128, [scl[j][:] for j in range(MT)], ACTF.Sigmoid, u_t)

        # h_new = r * h_sel; selc128 = [x (65) | hn (63)]; last hn channel is
        # the per-partition scalar
        selc128 = sing.tile([P, NQ], BF16, name="selc128")
        nc.scalar.copy(selc128[0:CX, :], combT[0:CX, 0:NQ])
        hnT = sing.tile([CH, NQ], BF16)
        hn_t = []
        for j in range(MT):
            hn = work.tile([P, D], F32, tag=f"hn{j}", name=f"hn{j}")
            nc.vector.tensor_tensor(hn[:], r_t[j][:], comb[j][:, CX:C], ALU.mult)
            hn_t.append(hn)
            pt = pA(CH, P)
            nc.tensor.transpose(pt[:], hn[:], identf[:])
            nc.scalar.copy(hnT[:, ts(j, P)], pt[:])
        nc.gpsimd.dma_start(selc128[CX:P, :], hnT[0:CH - 1, :])
        hyper(2, selc128, [hn_t[j][:, D - 1:D] for j in range(MT)],
              ACTF.Tanh, c_t)

        # out = h_new + u * (cand - h_new)
        for j in range(MT):
            t1 = work.tile([P, D], F32, tag="t1", name="t1")
            nc.vector.tensor_tensor(t1[:], c_t[j][:], hn_t[j][:], ALU.subtract)
            nc.vector.tensor_tensor(t1[:], t1[:], u_t[j][:], ALU.mult)
            nc.vector.tensor_tensor(t1[:], t1[:], hn_t[j][:], ALU.add)
            nc.sync.dma_start(out[ts(j, P), :], t1[:])

    return nc


_NC_CACHE = None


def _get_nc():
    global _NC_CACHE
    if _NC_CACHE is None:
        _NC_CACHE = build_graph()
        if not _NC_CACHE.is_finalized():
            _NC_CACHE.finalize()
    return _NC_CACHE


def _prep_core(b, x, h, query_vectors, adj, nodes_n, shared):
    idx = nodes_n[b * NQ:(b + 1) * NQ].astype(np.int64)
    rest = np.setdiff1d(np.arange(N, dtype=np.int64), idx)
    perm = np.concatenate([idx, rest])
    d = dict(shared)
    d["xp"] = np.ascontiguousarray(x[b][perm])
    d["hp"] = np.ascontiguousarray(h[b][perm])
    d["adjT"] = np.ascontiguousarray(
        np.where(adj[np.ix_(idx, perm)] != 0, np.float32(0), np.float32(NEG)).T)
    d["qv"] = np.ascontiguousarray(query_vectors[b * NQ:(b + 1) * NQ])
    return d


def _prep_shared(Wq, bq, Wk, bk, Wv, bv, W1, b1, W2, b2, Wr, br, Wu, bu, Wc, bc):
    bf = ml_dtypes.bfloat16
    W1r = np.asarray(W1, np.float32).reshape(H, C, C)
    b1_eff = np.asarray(b1, np.float32) + sum(
        np.asarray(bv, np.float32)[hh] @ W1r[hh] for hh in range(H))
    # per-head [C, 48]: k at 0:16, q/4 at 32:48; bias likewise
    wqk_np = np.zeros((H, C, 48), np.float32)
    wqk_np[:, :, 0:16] = np.asarray(Wk, np.float32)
    wqk_np[:, :, 32:48] = np.asarray(Wq, np.float32) * 0.25
    bqk_np = np.zeros((H, 48), np.float32)
    bqk_np[:, 0:16] = np.asarray(bk, np.float32)
    bqk_np[:, 32:48] = np.asarray(bq, np.float32) * 0.25
    packW = lambda W: np.ascontiguousarray(
        np.transpose(np.asarray(W, np.float32), (1, 2, 0)).reshape(C, D * QV)[0:P])
    lastW = lambda W: np.ascontiguousarray(np.asarray(W, np.float32)[:, P, :])
    return dict(
        wqk=np.ascontiguousarray(wqk_np.astype(bf)),
        bqk=np.ascontiguousarray(bqk_np),
        wv=np.ascontiguousarray(np.asarray(Wv, np.float32).astype(bf)),
        w1=np.ascontiguousarray(W1r.astype(bf)),
        b1=np.ascontiguousarray(b1_eff),
        w2=np.ascontiguousarray(np.asarray(W2, np.float32).astype(bf)),
        b2=np.ascontiguousarray(np.asarray(b2, np.float32)),
        wall=np.ascontiguousarray(np.stack(
            [packW(Wr), packW(Wu), packW(Wc)]).astype(bf)),
        w128=np.ascontiguousarray(np.stack(
            [lastW(Wr), lastW(Wu), lastW(Wc)]).astype(bf)),
        ball=np.ascontiguousarray(np.stack([
            np.asarray(br, np.float32), np.asarray(bu, np.float32),
            np.asarray(bc, np.float32)]).astype(bf)),
    )


def make_in_maps(x, h, query_vectors, adj, nodes_b, nodes_n, **weights):
    x = np.asarray(x, np.float32)
    h = np.asarray(h, np.float32)
    query_vectors = np.asarray(query_vectors, np.float32)
    adj = np.asarray(adj)
    nodes_n = np.asarray(nodes_n)
    shared = _prep_shared(**weights)
    return [_prep_core(b, x, h, query_vectors, adj, nodes_n, shared)
            for b in range(B)]


def kernel(x, h, query_vectors, adj, nodes_b, nodes_n,
           Wq, bq, Wk, bk, Wv, bv, W1, b1, W2, b2,
           Wr, br, Wu, bu, Wc, bc):
    in_maps = make_in_maps(
        x, h, query_vectors, adj, nodes_b, nodes_n,
        Wq=Wq, bq=bq, Wk=Wk, bk=bk, Wv=Wv, bv=bv, W1=W1, b1=b1, W2=W2, b2=b2,
        Wr=Wr, br=br, Wu=Wu, bu=bu, Wc=Wc, bc=bc)
    nc = _get_nc()
    res = run_bass_kernel_spmd(nc, in_maps, list(range(B)))
    outs = [np.asarray(res.results[b]["out"], np.float32) for b in range(B)]
    return np.concatenate(outs, axis=0)


# revision 16
# speedup vs baseline: 1.0198x; 1.0198x over previous
"""AGATCellWithMLP Trainium2 kernel: 8-core data-parallel over batch B.

Each core handles one graph. Host-side we permute each graph's nodes so the
512 "selected" nodes (nodes_n order) come first; the kernel then only computes
attention/MLP query rows for those 512 nodes (keys/values span all 1024).
All heavy tensors are kept in transposed [channel, node] layout so the
attention probabilities never need an on-chip transpose; the softmax
denominator is produced by an extra ones-column in the V matmul.

The combined-feature dim C=129 is split as 128 PE-contraction rows plus a
rank-1 "channel 128" correction (fused DVE op or tiny extra K-chunk), so
every big matmul needs only ONE K-chunk instead of two.
"""

import sys

sys.path.insert(0, "/opt/trn_rl_repo")

from contextlib import ExitStack

import numpy as np
import ml_dtypes

import concourse.bass as bass
import concourse.bacc as bacc
import concourse.tile as tile
from concourse import mybir
from concourse.bass_utils import run_bass_kernel_spmd
from concourse.masks import make_identity
from concourse.bass import ts

P = 128
B, N, D, H, QV = 8, 1024, 64, 4, 32
C = 2 * D + 1           # 129
KD = C // 8             # 16
NQ = 512                # selected nodes (queries) per graph
CX, CH = D + 1, D       # 65 + 64 channel split (x | h)
NEG = -9e15
F32 = mybir.dt.float32
BF16 = mybir.dt.bfloat16
AX = mybir.AxisListType
ALU = mybir.AluOpType
ACTF = mybir.ActivationFunctionType

NT = N // P             # 8 key tiles
MT = NQ // P            # 4 query tiles


def build_graph():
    nc = bacc.Bacc()

    xp = nc.declare_dram_parameter("xp", [N, CX], F32, False)
    hp = nc.declare_dram_parameter("hp", [N, CH], F32, False)
    adjT = nc.declare_dram_parameter("adjT", [N, NQ], F32, False)
    qv = nc.declare_dram_parameter("qv", [NQ, QV], F32, False)
    # per-head q|k weights packed [C, 48]: k at cols 0:16, q (pre-scaled) at 32:48
    wqk = nc.declare_dram_parameter("wqk", [H, C, 48], BF16, False)
    bqk = nc.declare_dram_parameter("bqk", [H, 48], F32, False)
    wv = nc.declare_dram_parameter("wv", [H, C, C], BF16, False)
    w1 = nc.declare_dram_parameter("w1", [H, C, C], BF16, False)
    b1 = nc.declare_dram_parameter("b1", [C], F32, False)
    w2 = nc.declare_dram_parameter("w2", [C, C], BF16, False)
    b2 = nc.declare_dram_parameter("b2", [C], F32, False)
    wall = nc.declare_dram_parameter("wall", [3, P, QV * D], BF16, False)
    w128 = nc.declare_dram_parameter("w128", [3, QV, D], BF16, False)
    ball = nc.declare_dram_parameter("ball", [3, QV, D], BF16, False)
    out = nc.declare_dram_parameter("out", [NQ, D], F32, True)

    with tile.TileContext(nc) as tc, ExitStack() as ctx:
        sing = ctx.enter_context(tc.tile_pool(name="sing", bufs=1))
        work = ctx.enter_context(tc.tile_pool(name="work", bufs=3))
        pexp_pool = ctx.enter_context(tc.tile_pool(name="pexp", bufs=4))
        vpool = ctx.enter_context(tc.tile_pool(name="vpool", bufs=4))
        psA = ctx.enter_context(tc.tile_pool(name="psA", bufs=3, space="PSUM"))
        psB = ctx.enter_context(tc.tile_pool(name="psB", bufs=2, space="PSUM"))
        psC = ctx.enter_context(tc.tile_pool(name="psC", bufs=1, space="PSUM"))
        psS = ctx.enter_context(tc.tile_pool(name="psS", bufs=2, space="PSUM"))

        def pA(p_, f_):
            return psA.tile([p_, f_], F32, tag="pA", name="pA")

        def pAb(p_, f_):
            return psA.tile([p_, f_], BF16, tag="pA", name="pA")

        def pB(p_, f_):
            return psB.tile([p_, f_], F32, tag="pB", name="pB")

        def pC(p_, f_):
            return psC.tile([p_, f_], F32, tag="pC", name="pC")

        def pS(p_, f_):
            return psS.tile([p_, f_], F32, tag="pS", name="pS")

        ident = sing.tile([P, P], BF16)
        make_identity(nc, ident[:])
        identf = sing.tile([P, P], F32)
        make_identity(nc, identf[:])
        ones_row = sing.tile([1, P], BF16)
        nc.vector.memset(ones_row[:], 1.0)

        # ---- inputs on the gpsimd queue (parallel with weight DMAs) ----
        comb = [sing.tile([P, C], F32, tag=f"comb{i}", name=f"comb{i}")
                for i in range(NT)]
        for i in range(NT):
            nc.gpsimd.dma_start(comb[i][:, 0:CX], xp[ts(i, P), :])
            nc.gpsimd.dma_start(comb[i][:, CX:C], hp[ts(i, P), :])
        qv_sb = [sing.tile([P, QV], F32, tag=f"qv{j}", name=f"qv{j}")
                 for j in range(MT)]
        for j in range(MT):
            nc.gpsimd.dma_start(qv_sb[j][:], qv[ts(j, P), :])
        adjT_sb = [sing.tile([P, NQ], F32, tag=f"adjT{i}", name=f"adjT{i}")
                   for i in range(NT)]
        for i in range(NT):
            nc.gpsimd.dma_start(adjT_sb[i][:], adjT[ts(i, P), :])

        # ---- weights on the sync queue ----
        wqk_sb = [sing.tile([P, 48], BF16, tag=f"wqk{h}", name=f"wqk{h}")
                  for h in range(H)]
        wqkL = [sing.tile([1, 48], BF16, tag=f"wqkL{h}", name=f"wqkL{h}")
                for h in range(H)]
        wv_sb = [sing.tile([P, C], BF16, tag=f"wv{h}", name=f"wv{h}")
                 for h in range(H)]
        wvL = [sing.tile([1, C], BF16, tag=f"wvL{h}", name=f"wvL{h}")
               for h in range(H)]
        w1_sb = [sing.tile([P, C], BF16, tag=f"w1{h}", name=f"w1{h}")
                 for h in range(H)]
        w1L = sing.tile([H, C], BF16, name="w1L")
        for h in range(H):
            nc.sync.dma_start(wqk_sb[h][:], wqk[h, 0:P, :])
            nc.sync.dma_start(wqkL[h][:], wqk[h, P:C, :])
            nc.sync.dma_start(wv_sb[h][:], wv[h, 0:P, :])
            nc.sync.dma_start(wvL[h][:], wv[h, P:C, :])
            nc.sync.dma_start(w1_sb[h][:], w1[h, 0:P, :])
            nc.sync.dma_start(w1L[h:h + 1, :], w1[h, P:C, :])
        w2_sb = (sing.tile([CX, C], BF16, tag="w2x", name="w2x"),
                 sing.tile([CH, C], BF16, tag="w2h", name="w2h"))
        nc.sync.dma_start(w2_sb[0][:], w2[0:CX, :])
        nc.sync.dma_start(w2_sb[1][:], w2[CX:C, :])
        bqk_sb = [sing.tile([48, 1], F32, tag=f"bqk{h}", name=f"bqk{h}")
                  for h in range(H)]
        for h in range(H):
            nc.sync.dma_start(bqk_sb[h][:], bqk[h, :, None])
        b1_sb = (sing.tile([CX, 1], F32, tag="b1x", name="b1x"),
                 sing.tile([CH, 1], F32, tag="b1h", name="b1h"))
        nc.sync.dma_start(b1_sb[0][:], b1[0:CX, None])
        nc.sync.dma_start(b1_sb[1][:], b1[CX:C, None])
        b2_sb = (sing.tile([CX, 1], F32, tag="b2x", name="b2x"),
                 sing.tile([CH, 1], F32, tag="b2h", name="b2h"))
        nc.sync.dma_start(b2_sb[0][:], b2[0:CX, None])
        nc.sync.dma_start(b2_sb[1][:], b2[CX:C, None])
        wall_sb = [sing.tile([P, QV * D], BF16, tag=f"wall{i}", name=f"wall{i}")
                   for i in range(3)]
        w128_sb = [sing.tile([QV, D], BF16, tag=f"w128_{i}", name=f"w128_{i}")
                   for i in range(3)]
        ball_sb = [sing.tile([QV, D], BF16, tag=f"ball{i}", name=f"ball{i}")
                   for i in range(3)]
        for i in range(3):
            nc.sync.dma_start(wall_sb[i][:], wall[i, :, :])
            nc.sync.dma_start(w128_sb[i][:], w128[i, :, :])
            nc.sync.dma_start(ball_sb[i][:], ball[i, :, :])

        # ---- transposed combined: combT [128, 1024] (c 0:128) + c128T [1, 1024]
        combT = sing.tile([P, N], BF16)
        c128T = sing.tile([1, N], BF16)
        for i in range(NT):
            cb = work.tile([P, C], BF16, tag="cb", name="cb")
            nc.scalar.copy(cb[:], comb[i][:])
            pt = pAb(P, P)
            nc.tensor.transpose(pt[:], cb[:, 0:P], ident[:])
            nc.scalar.copy(combT[:, ts(i, P)], pt[:])
            pl = pAb(1, P)
            nc.tensor.transpose(pl[:], cb[:, P:C], ident[:])
            nc.scalar.copy(c128T[:, ts(i, P)], pl[:])
        # qv transposed [32, 512]
        qvT = sing.tile([QV, NQ], BF16)
        for j in range(MT):
            qb16 = work.tile([P, QV], BF16, tag="qb16", name="qb16")
            nc.scalar.copy(qb16[:], qv_sb[j][:])
            pq = pAb(QV, P)
            nc.tensor.transpose(pq[:], qb16[:], ident[:])
            nc.scalar.copy(qvT[:, ts(j, P)], pq[:])

        # residual rows for the cf h-group: channels 65:129 = combT[65:128]+c128T
        combresH = sing.tile([CH, NQ], BF16)
        nc.gpsimd.dma_start(combresH[0:CH - 1, :], combT[CX:P, 0:NQ])
        nc.gpsimd.dma_start(combresH[CH - 1:CH, :], c128T[:, 0:NQ])

        # ---- attention heads ----
        acT = []
        for h in range(H):
            # q,k in one matmul pair: psum rows 0:16 = k, rows 32:48 = q
            kT = work.tile([KD, N], BF16, tag="kT", name="kT")
            qT = work.tile([KD, NQ], BF16, tag="qT", name="qT")
            for half in range(2):
                pk = pC(48, NQ)
                nc.tensor.matmul(pk[:], wqk_sb[h][:], combT[:, ts(half, NQ)],
                                 start=True, stop=False)
                nc.tensor.matmul(pk[:], wqkL[h][:], c128T[:, ts(half, NQ)],
                                 start=False, stop=True)
                nc.scalar.activation(kT[:, ts(half, NQ)], pk[0:KD, :],
                                     ACTF.Identity, bias=bqk_sb[h][0:KD, :])
                if half == 0:
                    nc.scalar.activation(qT[:], pk[32:48, :], ACTF.Identity,
                                         bias=bqk_sb[h][32:48, :])

            # broadcast Wv's channel-128 row across partitions (for the V fixup)
            pvl = pA(P, C)
            nc.tensor.matmul(pvl[:], ones_row[:], wvL[h][:], start=True, stop=True)
            wvLrep = work.tile([P, C], BF16, tag="wvLrep", name="wvLrep")
            nc.vector.tensor_copy(wvLrep[:], pvl[:])

            phg1 = pB(CX, NQ)
            phg2 = pB(CX, NQ)
            for i in range(NT):
                # V tile [128, 130]: cols 0:129 = combined @ Wv, col 129 = 1.0
                vt = vpool.tile([P, C + 1], BF16, tag="vt", name="vt")
                pv = pA(P, C)
                nc.tensor.matmul(pv[:], combT[:, ts(i, P)], wv_sb[h][:],
                                 start=True, stop=True)
                # channel-128 rank-1 fixup fused with the PSUM->SBUF copy
                nc.vector.scalar_tensor_tensor(
                    vt[:, 0:C], wvLrep[:], comb[i][:, P:C], pv[:],
                    op0=ALU.mult, op1=ALU.add)
                nc.gpsimd.memset(vt[:, C:C + 1], 1.0)

                # scores^T tile: [128 keys, 512 queries]
                ps = pS(P, NQ)
                nc.tensor.matmul(ps[:], kT[:, ts(i, P)], qT[:],
                                 start=True, stop=True)
                # mask first (leaky(s+m) == leaky(s)+m for m in {0, -9e15}),
                # then leaky = max(t, 0.2t) in one fused DVE op, then exp.
                t0 = work.tile([P, NQ], F32, tag="t0", name="t0")
                nc.vector.tensor_tensor(t0[:], ps[:], adjT_sb[i][:], ALU.add)
                sm = work.tile([P, NQ], F32, tag="sm", name="sm")
                nc.vector.scalar_tensor_tensor(sm[:], t0[:], 0.2, t0[:],
                                               op0=ALU.mult, op1=ALU.max)
                pe = pexp_pool.tile([P, NQ], BF16, tag="pe", name="pe")
                nc.scalar.activation(pe[:], sm[:], ACTF.Exp)

                nc.tensor.matmul(phg1[:], vt[:, 0:CX], pe[:],
                                 start=(i == 0), stop=(i == NT - 1))
                nc.tensor.matmul(phg2[:], vt[:, CX:C + 1], pe[:],
                                 start=(i == 0), stop=(i == NT - 1))

            # stash raw hp (and the denominator row); normalization is batched
            ar1 = sing.tile([CX, NQ], F32, tag=f"araw{h}x", name=f"araw{h}x")
            ar2 = sing.tile([CX, NQ], F32, tag=f"araw{h}h", name=f"araw{h}h")
            nc.scalar.copy(ar1[:], phg1[:])
            nc.scalar.copy(ar2[:], phg2[:])
            acT.append((ar1, ar2))

        # ---- batched softmax normalization: ONE reciprocal for all 4 heads.
        # Engine reads/writes need base partition 0/32/64, so row scatter and
        # gather go through tiny SBUF-SBUF DMAs.
        rcat = sing.tile([H, NQ], F32, name="rcat")
        for h in range(H):
            nc.gpsimd.dma_start(rcat[h:h + 1, :], acT[h][1][CX - 1:CX, :])
        rinv4 = sing.tile([H, NQ], F32, name="rinv4")
        nc.vector.reciprocal(rinv4[:], rcat[:])
        rtmp = [sing.tile([1, NQ], F32, tag=f"rtmp{h}", name=f"rtmp{h}")
                for h in range(H)]
        for h in range(H):
            nc.gpsimd.dma_start(rtmp[h][:], rinv4[h:h + 1, :])
        # normalized attn_cat^T per head: a128 [128, 512] (c 0:128) in one tile
        # (rows 65:128 arrive via DMA), last channels gathered into aL [4, 512]
        a128 = []
        aL = sing.tile([H, NQ], BF16, name="aL")
        for h in range(H):
            ar1, ar2 = acT[h]
            rinv_bf = work.tile([1, NQ], BF16, tag="rinv_bf", name="rinv_bf")
            nc.scalar.copy(rinv_bf[:], rtmp[h][:])
            pbc = pA(P, NQ)
            nc.tensor.matmul(pbc[:], ones_row[:], rinv_bf[:], start=True, stop=True)
            rb = work.tile([P, NQ], F32, tag="rb", name="rb")
            nc.scalar.copy(rb[:], pbc[:])
            ah = sing.tile([P, NQ], BF16, tag=f"a128_{h}", name=f"a128_{h}")
            nc.vector.tensor_tensor(ah[0:CX, :], ar1[:], rb[0:CX, :], ALU.mult)
            a2t = sing.tile([CH, NQ], BF16, tag=f"a2t{h}", name=f"a2t{h}")
            nc.vector.tensor_tensor(a2t[:], ar2[0:CH, :], rb[0:CH, :], ALU.mult)
            nc.gpsimd.dma_start(ah[CX:P, :], a2t[0:CH - 1, :])
            nc.gpsimd.dma_start(aL[h:h + 1, :], a2t[CH - 1:CH, :])
            a128.append(ah)

        # ---- MLP (transposed): m1 = relu(W1^T ac + b1); cf = W2^T m1 + b2 + comb
        m1T = (work.tile([CX, NQ], BF16, tag="m1x", name="m1x"),
               work.tile([CH, NQ], BF16, tag="m1h", name="m1h"))
        for g, (off, ln) in enumerate(((0, CX), (CX, CH))):
            pm = pB(CX, NQ)
            for h in range(H):
                nc.tensor.matmul(pm[0:ln, :], w1_sb[h][:, off:off + ln],
                                 a128[h][:], start=(h == 0), stop=False)
            nc.tensor.matmul(pm[0:ln, :], w1L[:, off:off + ln], aL[:],
                             start=False, stop=True)
            nc.scalar.activation(m1T[g][:], pm[0:ln, :], ACTF.Relu, bias=b1_sb[g][:])
        # cf128 [128, 512] = combined_final^T channels 0:128; cl_row = channel 128
        cf128 = sing.tile([P, NQ], BF16, name="cf128")
        cfh = work.tile([CH, NQ], BF16, tag="cfh", name="cfh")
        for g, (off, ln) in enumerate(((0, CX), (CX, CH))):
            pm = pB(CX, NQ)
            nc.tensor.matmul(pm[0:ln, :], w2_sb[0][:, off:off + ln], m1T[0][:],
                             start=True, stop=False)
            nc.tensor.matmul(pm[0:ln, :], w2_sb[1][:, off:off + ln], m1T[1][:],
                             start=False, stop=True)
            dst = cf128[0:CX, :] if g == 0 else cfh[:]
            src = combT[0:CX, 0:NQ] if g == 0 else combresH[:]
            nc.vector.scalar_tensor_tensor(dst, pm[0:ln, :], b2_sb[g][:], src,
                                           op0=ALU.add, op1=ALU.add)
        nc.gpsimd.dma_start(cf128[CX:P, :], cfh[0:CH - 1, :])
        cl_row = sing.tile([1, NQ], BF16, name="cl_row")
        nc.gpsimd.dma_start(cl_row[:], cfh[CH - 1:CH, :])
        # natural-layout channel-128 scalars [128, 1] per query tile
        scl = [sing.tile([P, 1], F32, tag=f"scl{j}", name=f"scl{j}")
               for j in range(MT)]
        for j in range(MT):
            pt = pAb(P, 1)
            nc.tensor.transpose(pt[:], cl_row[:, ts(j, P)], ident[0:1, 0:1])
            nc.scalar.copy(scl[j][:], pt[:])

        # ---- hypernetwork stage ----
        def hyper(idx, sel128, selL, func, outs):
            """outs[j][128,64] = func(sel @ W_all + selL*(qv@W128) + qv @ b).

            wall[c, o*QV+d] = W[d, c, o] for c<128; the c=128 row is handled
            via the per-partition scalar selL and qv @ W128.
            """
            og = NQ // QV  # o-values per 512-wide chunk (16)
            for j in range(MT):
                pbn = pC(P, 2 * D)
                nc.tensor.matmul(pbn[:, 0:D], qvT[:, ts(j, P)], ball_sb[idx][:],
                                 start=True, stop=True)
                nc.tensor.matmul(pbn[:, D:2 * D], qvT[:, ts(j, P)],
                                 w128_sb[idx][:], start=True, stop=True)
                o1 = work.tile([P, D], F32, tag="o1", name="o1")
                for nch in range(4):  # 512-wide chunks of the 2048 (o,d) axis
                    pt = pA(P, NQ)
                    nc.tensor.matmul(pt[:], sel128[:, ts(j, P)],
                                     wall_sb[idx][:, ts(nch, NQ)],
                                     start=True, stop=True)
                    prod = work.tile([P, NQ], BF16, tag="prod", name="prod")
                    qb = qv_sb[j][:, None, :].to_broadcast((P, og, QV))
                    nc.vector.tensor_tensor(
                        prod[:].rearrange("p (a b) -> p a b", b=QV),
                        pt[:].rearrange("p (a b) -> p a b", b=QV),
                        qb, ALU.mult)
                    nc.vector.tensor_reduce(
                        o1[:, ts(nch, og)],
                        prod[:].rearrange("p (a b) -> p a b", b=QV),
                        axis=AX.X, op=ALU.add)
                nc.vector.tensor_tensor(o1[:], o1[:], pbn[:, 0:D], ALU.add)
                nc.vector.scalar_tensor_tensor(o1[:], pbn[:, D:2 * D], selL[j],
                                               o1[:], op0=ALU.mult, op1=ALU.add)
                nc.scalar.activation(outs[j][:], o1[:], func)

        r_t = [work.tile([P, D], F32, tag=f"r{j}", name=f"r{j}") for j in range(MT)]
        u_t = [work.tile([P, D], F32, tag=f"u{j}", name=f"u{j}") for j in range(MT)]
        c_t = [work.tile([P, D], F32, tag=f"c{j}", name=f"c{j}") for j in range(MT)]
        hyper(0, cf128, [scl[j][:] for j in range(MT)], ACTF.Sigmoid, r_t)
        hyper(1, cf128, [scl[j][:] for j in range(MT)], ACTF.Sigmoid, u_t)

        # h_new = r * h_sel; selc128 = [x (65) | hn (63)]; last hn channel is
        # the per-partition scalar
        selc128 = sing.tile([P, NQ], BF16, name="selc128")
        nc.scalar.copy(selc128[0:CX, :], combT[0:CX, 0:NQ])
        hnT = sing.tile([CH, NQ], BF16)
        hn_t = []
        for j in range(MT):
            hn = work.tile([P, D], F32, tag=f"hn{j}", name=f"hn{j}")
            nc.vector.tensor_tensor(hn[:], r_t[j][:], comb[j][:, CX:C], ALU.mult)
            hn_t.append(hn)
            pt = pA(CH, P)
            nc.tensor.transpose(pt[:], hn[:], identf[:])
            nc.scalar.copy(hnT[:, ts(j, P)], pt[:])
        nc.gpsimd.dma_start(selc128[CX:P, :], hnT[0:CH - 1, :])
        hyper(2, selc128, [hn_t[j][:, D - 1:D] for j in range(MT)],
              ACTF.Tanh, c_t)

        # out = h_new + u * (cand - h_new)
        for j in range(MT):
            t1 = work.tile([P, D], F32, tag="t1", name="t1")
            nc.vector.tensor_tensor(t1[:], c_t[j][:], hn_t[j][:], ALU.subtract)
            nc.vector.tensor_tensor(t1[:], t1[:], u_t[j][:], ALU.mult)
            nc.vector.tensor_tensor(t1[:], t1[:], hn_t[j][:], ALU.add)
            nc.sync.dma_start(out[ts(j, P), :], t1[:])

    return nc


_NC_CACHE = None


def _get_nc():
    global _NC_CACHE
    if _NC_CACHE is None:
        _NC_CACHE = build_graph()
        if not _NC_CACHE.is_finalized():
            _NC_CACHE.finalize()
    return _NC_CACHE


def _prep_core(b, x, h, query_vectors, adj, nodes_n, shared):
    idx = nodes_n[b * NQ:(b + 1) * NQ].astype(np.int64)
    rest = np.setdiff1d(np.arange(N, dtype=np.int64), idx)
    perm = np.concatenate([idx, rest])
    d = dict(shared)
    d["xp"] = np.ascontiguousarray(x[b][perm])
    d["hp"] = np.ascontiguousarray(h[b][perm])
    d["adjT"] = np.ascontiguousarray(
        np.where(adj[np.ix_(idx, perm)] != 0, np.float32(0), np.float32(NEG)).T)
    d["qv"] = np.ascontiguousarray(query_vectors[b * NQ:(b + 1) * NQ])
    return d


def _prep_shared(Wq, bq, Wk, bk, Wv, bv, W1, b1, W2, b2, Wr, br, Wu, bu, Wc, bc):
    bf = ml_dtypes.bfloat16
    W1r = np.asarray(W1, np.float32).reshape(H, C, C)
    b1_eff = np.asarray(b1, np.float32) + sum(
        np.asarray(bv, np.float32)[hh] @ W1r[hh] for hh in range(H))
    # per-head [C, 48]: k at 0:16, q/4 at 32:48; bias likewise
    wqk_np = np.zeros((H, C, 48), np.float32)
    wqk_np[:, :, 0:16] = np.asarray(Wk, np.float32)
    wqk_np[:, :, 32:48] = np.asarray(Wq, np.float32) * 0.25
    bqk_np = np.zeros((H, 48), np.float32)
    bqk_np[:, 0:16] = np.asarray(bk, np.float32)
    bqk_np[:, 32:48] = np.asarray(bq, np.float32) * 0.25
    packW = lambda W: np.ascontiguousarray(
        np.transpose(np.asarray(W, np.float32), (1, 2, 0)).reshape(C, D * QV)[0:P])
    lastW = lambda W: np.ascontiguousarray(np.asarray(W, np.float32)[:, P, :])
    return dict(
        wqk=np.ascontiguousarray(wqk_np.astype(bf)),
        bqk=np.ascontiguousarray(bqk_np),
        wv=np.ascontiguousarray(np.asarray(Wv, np.float32).astype(bf)),
        w1=np.ascontiguousarray(W1r.astype(bf)),
        b1=np.ascontiguousarray(b1_eff),
        w2=np.ascontiguousarray(np.asarray(W2, np.float32).astype(bf)),
        b2=np.ascontiguousarray(np.asarray(b2, np.float32)),
        wall=np.ascontiguousarray(np.stack(
            [packW(Wr), packW(Wu), packW(Wc)]).astype(bf)),
        w128=np.ascontiguousarray(np.stack(
            [lastW(Wr), lastW(Wu), lastW(Wc)]).astype(bf)),
        ball=np.ascontiguousarray(np.stack([
            np.asarray(br, np.float32), np.asarray(bu, np.float32),
            np.asarray(bc, np.float32)]).astype(bf)),
    )


def make_in_maps(x, h, query_vectors, adj, nodes_b, nodes_n, **weights):
    x = np.asarray(x, np.float32)
    h = np.asarray(h, np.float32)
    query_vectors = np.asarray(query_vectors, np.float32)
    adj = np.asarray(adj)
    nodes_n = np.asarray(nodes_n)
    shared = _prep_shared(**weights)
    return [_prep_core(b, x, h, query_vectors, adj, nodes_n, shared)
            for b in range(B)]


def kernel(x, h, query_vectors, adj, nodes_b, nodes_n,
           Wq, bq, Wk, bk, Wv, bv, W1, b1, W2, b2,
           Wr, br, Wu, bu, Wc, bc):
    in_maps = make_in_maps(
        x, h, query_vectors, adj, nodes_b, nodes_n,
        Wq=Wq, bq=bq, Wk=Wk, bk=bk, Wv=Wv, bv=bv, W1=W1, b1=b1, W2=W2, b2=b2,
        Wr=Wr, br=br, Wu=Wu, bu=bu, Wc=Wc, bc=bc)
    nc = _get_nc()
    res = run_bass_kernel_spmd(nc, in_maps, list(range(B)))
    outs = [np.asarray(res.results[b]["out"], np.float32) for b in range(B)]
    return np.concatenate(outs, axis=0)
